# revision 44
# baseline (speedup 1.0000x reference)
"""Trainium2 Bass kernel for fused attention (QKV proj + RoPE + SDPA + o_proj).

Sharding: Megatron-style tensor parallel over heads (4 heads/core x 8 cores)
for QKV+SDPA, then per-(batch, query-half) AllToAll rounds switch to token
parallelism for o_proj, so each core emits a disjoint slice of the output.

v4 design (bf16 everywhere; fp8 was measured to blow the 2e-2 gate):
  - all matmuls bf16 (psum accumulate f32); w_o resident in SBUF
  - ap=512 moving rows everywhere; RoPE batched per [128,512] tile
  - per-batch AllToAll rounds + last batch split into qt halves; a
    full-size warmup collective absorbs stream/first-transfer setup
  - o_proj matmuls interleave into SDPA slots, gated late enough that
    their A2A has landed (a waiting matmul head-of-line blocks the PE
    FIFO); round 1 is held out of sdpa(b2) and fed to sdpa(b3), which
    has no projection left and is otherwise exp-latency-bound
  - queue discipline: gpsimd = collective triggers + asl loads (each
    asl load emitted directly behind its trigger); scalar = exp + out
    stores + half the bulk loads; sync = rope swaps, dg/rd/rep chain
    (same-queue: cross-queue DRAM deps race), scatters, other bulk;
    hid prefetches diced into 512KB chunks emitted at buffer-free time
  - dummy matmuls on a memset tile warm the PE clock (HAM) through the
    startup DMA window
"""
import sys

import numpy as np

try:
    import concourse.bass as bass
except ImportError:  # fresh grading env: make the toolchain importable
    for p in (
        "/root/.axon_site",
        "/root/.axon_site/_ro/trn_rl_repo",
        "/root/.axon_site/_ro/pypackages",
        "/opt/trn_rl_repo",
        "/opt/pypackages",
    ):
        if p not in sys.path:
            sys.path.append(p)
    import concourse.bass as bass

import concourse.bacc as bacc
import concourse.mybir as mybir
import concourse.tile as tile
from concourse.bass_utils import run_bass_kernel_spmd

import ml_dtypes

F32 = mybir.dt.float32
F32R = mybir.dt.float32r
BF16 = mybir.dt.bfloat16
MULT = mybir.AluOpType.mult
ADD = mybir.AluOpType.add
EXP = mybir.ActivationFunctionType.Exp
IDENT = mybir.ActivationFunctionType.Identity

# problem dims (hardcoded for nn_Attention_42846593744909)
B, S, D = 4, 1024, 2048
H, HD = 32, 64
N_CORES = 8
H_LOC = H // N_CORES  # heads per core


def build_attention(b=B, s=S, d=D, h_loc=H_LOC, hd=HD, n_cores=N_CORES):
    """Build the per-core SPMD Bass program. Returns finalized nc."""
    P = 128
    T = b * s                  # total tokens
    DCH = d // P               # contraction chunks for D (16)
    QBLK = h_loc * hd          # 256
    NQK = 2 * QBLK // P        # q+k e-chunks (4)
    EVA = h_loc * (hd + 1)     # v + ones columns (260)
    TH = 512                   # proj token half-batch
    NTH = s // TH              # 2
    QT = 512                   # query tile in SDPA
    NQT = s // QT              # 2
    KTC = s // P               # key chunks of 128 (8)
    ECH = n_cores * QBLK // P  # o_proj contraction chunks (16)
    RT = s // n_cores          # tokens per core per batch round (128)
    ODC = 512                  # o_proj dout chunk (psum bank)
    NDC = d // ODC             # 4
    TS = b * RT                # output tokens per core (512)

    nc = bacc.Bacc()
    hidden_t = nc.dram_tensor("hidden_t", [d, T], BF16, kind="ExternalInput")
    w_qk_t = nc.dram_tensor("w_qk_t", [d, 2 * QBLK], BF16, kind="ExternalInput")
    w_v_t = nc.dram_tensor("w_v_t", [d, QBLK], BF16, kind="ExternalInput")
    w_o_t = nc.dram_tensor("w_o_t", [n_cores * QBLK, d], BF16, kind="ExternalInput")
    cos2 = nc.dram_tensor("cos2", [P, s], BF16, kind="ExternalInput")
    sinrot2 = nc.dram_tensor("sinrot2", [P, s], BF16, kind="ExternalInput")
    out_sl = nc.dram_tensor("out_sl", [TS, d], F32, kind="ExternalOutput")

    hid_v = hidden_t[:].rearrange("(c p) t -> p c t", p=P)
    wqk_v = w_qk_t[:].rearrange("(c p) e -> p c e", p=P)
    wv_v = w_v_t[:].rearrange("(c p) e -> p c e", p=P)
    wo_v = w_o_t[:].rearrange("(c p) e -> p c e", p=P)

    with tile.TileContext(nc) as tc:
        with tc.tile_pool(name="dramp", bufs=1, space="DRAM") as dramp:
            # full-size warmup exchange: the first large A2A pays a one-time
            # stream-setup cost (~15us extra), absorb it off-critical-path
            ccw_in = dramp.tile([n_cores, QBLK, RT], BF16, name="ccw_in")
            ccw_out = dramp.tile([n_cores, QBLK, RT], BF16, name="ccw_out")
            cc_in = [dramp.tile([n_cores, QBLK, RT], BF16, name=f"cc_in_{r}")
                     for r in range(b - 1)]
            cc_out = [dramp.tile([n_cores, QBLK, RT], BF16, name=f"cc_out_{r}")
                      for r in range(b - 1)]
            # last batch split into qt halves for a shorter tail
            cc3_in = [dramp.tile([n_cores, QBLK, RT // 2], BF16, name=f"cc3_in_{q}")
                      for q in range(NQT)]
            cc3_out = [dramp.tile([n_cores, QBLK, RT // 2], BF16, name=f"cc3_out_{q}")
                       for q in range(NQT)]
            # scatter view: [pp, h, v, dst core j, t]
            ccin_v = [t_[:].rearrange("j (pp h v) t -> pp h v j t", pp=2, h=2, v=hd)
                      for t_ in cc_in]
            ccout_v = [t_[:].rearrange("j (ci p) t -> p (j ci) t", p=P)
                       for t_ in cc_out]
            cc3in_v = [t_[:].rearrange("j (pp h v) t -> pp h v j t", pp=2, h=2, v=hd)
                       for t_ in cc3_in]
            cc3out_v = [t_[:].rearrange("j (ci p) t -> p (j ci) t", p=P)
                        for t_ in cc3_out]

            import contextlib
            with contextlib.ExitStack() as _st:
                tabs = _st.enter_context(tc.tile_pool(name="tabs", bufs=1))
                hidp = _st.enter_context(tc.tile_pool(name="hidp", bufs=2))
                qkp = _st.enter_context(tc.tile_pool(name="qkp", bufs=2))
                vp = _st.enter_context(tc.tile_pool(name="vp", bufs=2))
                ropep = _st.enter_context(tc.tile_pool(name="ropep", bufs=2))
                expp = _st.enter_context(tc.tile_pool(name="expp", bufs=2))
                aop = _st.enter_context(tc.tile_pool(name="aop", bufs=2))
                dgp = _st.enter_context(tc.tile_pool(name="dgp", bufs=1))
                repp = _st.enter_context(tc.tile_pool(name="repp", bufs=1))
                aonp = _st.enter_context(tc.tile_pool(name="aonp", bufs=1))
                aslp = _st.enter_context(tc.tile_pool(name="aslp", bufs=2))
                vfp = _st.enter_context(tc.tile_pool(name="vfp", bufs=2))
                obp = _st.enter_context(tc.tile_pool(name="obp", bufs=1))
                drowp = _st.enter_context(tc.tile_pool(name="drowp", bufs=4, space="DRAM"))
                psP = _st.enter_context(tc.tile_pool(name="psP", bufs=2, space="PSUM"))
                psJ = _st.enter_context(tc.tile_pool(name="psJ", bufs=2, space="PSUM"))
                psS = _st.enter_context(tc.tile_pool(name="psS", bufs=2, space="PSUM"))
                psO = _st.enter_context(tc.tile_pool(name="psO", bufs=2, space="PSUM"))
                # ---- static tables / weights; issue order = priority.
                # Big loads spread across sync/scalar/gpsimd direct-DMA
                # queues so transfers run in parallel.
                wqk_sb = tabs.tile([P, DCH, 2 * QBLK], BF16)
                # warmup collective first: absorb cc stream setup off
                # the critical path
                nc.gpsimd.collective_compute(
                    "AllToAll", mybir.AluOpType.bypass,
                    replica_groups=[list(range(n_cores))],
                    ins=[ccw_in.opt()], outs=[ccw_out.opt()])
                nc.sync.dma_start(wqk_sb[:, :, 0:P], wqk_v[:, :, 0:P])
                # dummy matmuls (uninitialized operands, discarded psum):
                # keep the PE HAM activity window busy during the startup
                # DMA wait so real proj matmuls start at full clock
                wjunk = tabs.tile([P, ODC], BF16)
                nc.vector.memset(wjunk[:], 1.0)
                for _ in range(56):
                    pjw = psJ.tile([P, ODC], F32, tag="psJ", name="pjw")
                    nc.tensor.matmul(pjw[:], lhsT=wjunk[:, 0:P], rhs=wjunk[:],
                                     start=True, stop=True)

                def load_hid_half(bi, th, eng=None):
                    t0 = bi * s + th * TH
                    hid_sb = hidp.tile([P, DCH, TH], BF16, tag="hid", name="hid")
                    (eng or nc.sync).dma_start(hid_sb[:], hid_v[:, :, t0:t0 + TH])
                    return hid_sb

                cos_sb = tabs.tile([P, s], BF16)
                sin_sb = tabs.tile([P, s], BF16)
                wv_sb = tabs.tile([P, DCH, QBLK], BF16)
                # the critical first ~7MB is DMA-queue-serialization bound
                # (~100-200GB/s per queue), so split hid0/hid1 across both
                # hwdge queues; small wqk chunks slot between on sync
                hid0 = hidp.tile([P, DCH, TH], BF16, tag="hid", name="hid")
                hid1 = hidp.tile([P, DCH, TH], BF16, tag="hid", name="hid")
                nc.sync.dma_start(hid0[:, 0:8, :], hid_v[:, 0:8, 0:TH])
                nc.scalar.dma_start(hid0[:, 8:16, :], hid_v[:, 8:16, 0:TH])
                nc.scalar.dma_start(cos_sb[:], cos2[:])
                nc.scalar.dma_start(sin_sb[:], sinrot2[:])
                nc.scalar.dma_start(wv_sb[:], wv_v[:])
                nc.sync.dma_start(wqk_sb[:, :, P:2 * P], wqk_v[:, :, P:2 * P])
                for ec in range(2, NQK):
                    nc.sync.dma_start(wqk_sb[:, :, ec * P:(ec + 1) * P],
                                      wqk_v[:, :, ec * P:(ec + 1) * P])
                nc.sync.dma_start(hid1[:, 0:8, :], hid_v[:, 0:8, TH:2 * TH])
                nc.scalar.dma_start(hid1[:, 8:16, :], hid_v[:, 8:16, TH:2 * TH])
                hid_next = [hid0, hid1]
                wo_sb = tabs.tile([P, ECH, d], BF16)

                def rope(ps, soff, qk_t, ec):
                    """RoPE a [128, TH] psum tile into qk_t[:, ec, soff:soff+TH]."""
                    raw = ropep.tile([P, TH], F32, tag="raw", name="raw")
                    nc.vector.tensor_copy(raw[:], ps[:])
                    cp = ropep.tile([P, TH], F32, tag="cp", name="cp", bufs=1)
                    nc.vector.tensor_tensor(cp[:], raw[:], cos_sb[:, soff:soff + TH], MULT)
                    sw = ropep.tile([P, TH], F32, tag="sw", name="sw", bufs=1)
                    # rotate_half: swap 32-partition blocks within each head
                    nc.sync.dma_start(sw[0:32, :], raw[32:64, :])
                    nc.sync.dma_start(sw[32:64, :], raw[0:32, :])
                    nc.sync.dma_start(sw[64:96, :], raw[96:128, :])
                    nc.sync.dma_start(sw[96:128, :], raw[64:96, :])
                    nc.vector.tensor_tensor(sw[:], sw[:], sin_sb[:, soff:soff + TH], MULT)
                    nc.vector.tensor_tensor(qk_t[:, ec, soff:soff + TH], cp[:], sw[:], ADD)

                hid_store = {}

                def proj_gen(bi, qk_t, v_t):
                    """QKV projection + RoPE for batch bi, in 17 steps.
                    Next batch's hid prefetch is diced into 512KB chunks
                    emitted right when the pool buffer frees, so the sync
                    FIFO never blocks long (rope swap DMAs sit behind it)."""
                    hid_tiles = hid_store.pop(bi)
                    nxt = []
                    if bi + 1 < b:
                        nxt = [hidp.tile([P, DCH, TH], BF16, tag="hid",
                                         name="hid") for _ in range(NTH)]
                        hid_store[bi + 1] = nxt

                    def pref(th, g):
                        t0 = (bi + 1) * s + th * TH
                        nc.sync.dma_start(
                            nxt[th][:, 4 * g:4 * g + 4, :],
                            hid_v[:, 4 * g:4 * g + 4, t0:t0 + TH])

                    step = 0
                    for h in range(h_loc):
                        nc.scalar.activation(
                            v_t[:, :, h * (hd + 1) + hd:h * (hd + 1) + hd + 1],
                            wv_sb[:, 0:KTC, 0:1], IDENT, bias=1.0, scale=0.0)
                    yield
                    for th in range(NTH):
                        s0 = th * TH
                        hid_sb = hid_tiles[th]
                        for ec in range(NQK):
                            ps = psP.tile([P, ODC], F32, tag="psP", name="psqk")
                            for dd in range(DCH):
                                nc.tensor.matmul(
                                    ps[:], lhsT=wqk_sb[:, dd, ec * P:(ec + 1) * P],
                                    rhs=hid_sb[:, dd, :],
                                    start=(dd == 0), stop=(dd == DCH - 1))
                            rope(ps, s0, qk_t, ec)
                            step += 1
                            if nxt and 9 <= step + 1 <= 12:
                                pref(0, step + 1 - 9)
                            yield
                        for tsub in range(TH // P):
                            kc = th * (TH // P) + tsub
                            psv = psP.tile([P, ODC], F32, tag="psP", name="psv")
                            for dd in range(DCH):
                                nc.tensor.matmul(
                                    psv[:, 0:QBLK],
                                    lhsT=hid_sb[:, dd, tsub * P:(tsub + 1) * P],
                                    rhs=wv_sb[:, dd, :],
                                    start=(dd == 0), stop=(dd == DCH - 1))
                            for h in range(h_loc):
                                nc.vector.tensor_copy(
                                    v_t[:, kc, h * (hd + 1):h * (hd + 1) + hd],
                                    psv[:, h * hd:(h + 1) * hd])
                            step += 1
                            if nxt and 9 <= step + 1 <= 12:
                                pref(0, step + 1 - 9)
                            yield
                    # th1 buffer frees only now; its prefetch goes last
                    if nxt:
                        for g in range(4):
                            pref(1, g)

                def oproj_gen(r):
                    """Generator emitting o_proj for round r."""
                    asl = aslp.tile([P, ECH, RT], BF16, tag="asl", name=f"asl{r}")
                    nc.gpsimd.dma_start(asl[:], ccout_v[r])
                    yield
                    for dcg in range(NDC // 2):
                        pj0 = psJ.tile([P, ODC], F32, tag="psJ", name="pj0")
                        pj1 = psJ.tile([P, ODC], F32, tag="psJ", name="pj1")
                        d0 = (2 * dcg) * ODC
                        d1 = (2 * dcg + 1) * ODC
                        for e2 in range(ECH // 2):
                            for e in (2 * e2, 2 * e2 + 1):
                                nc.tensor.matmul(pj0[:], lhsT=asl[:, e, :],
                                                 rhs=wo_sb[:, e, d0:d0 + ODC],
                                                 start=(e == 0), stop=(e == ECH - 1))
                                nc.tensor.matmul(pj1[:], lhsT=asl[:, e, :],
                                                 rhs=wo_sb[:, e, d1:d1 + ODC],
                                                 start=(e == 0), stop=(e == ECH - 1))
                            yield
                        ob = obp.tile([P, 2 * ODC], F32, tag="ob", name="ob")
                        nc.vector.tensor_copy(ob[:, 0:ODC], pj0[:])
                        nc.vector.tensor_copy(ob[:, ODC:2 * ODC], pj1[:])
                        # store on gpsimd: with the current feed gating all
                        # stores emit after any collective trigger they
                        # could block, and this keeps the scalar FIFO clear
                        # for the exp cadence that paces b3's SDPA
                        nc.gpsimd.dma_start(
                            out_sl[r * RT:(r + 1) * RT, d0:d0 + 2 * ODC], ob[:])
                        yield

                def a2a(r):
                    nc.gpsimd.collective_compute(
                        "AllToAll", mybir.AluOpType.bypass,
                        replica_groups=[list(range(n_cores))],
                        ins=[cc_in[r].opt()], outs=[cc_out[r].opt()])

                def sdpa(bi, qk_t, v_t, feeders, pgen):
                    """SDPA for batch bi; o_proj rounds and the projection
                    of batch bi+1 interleave into the slots.  feeders is a
                    list of (gen, from_slot, every): from_slot must be late
                    enough that the round's A2A (~22us after trigger) has
                    landed, else the waiting o_proj matmul head-of-line
                    blocks the PE queue."""
                    slot = 0
                    for qt in range(NQT):
                        q0 = qt * QT
                        for pp in range(h_loc // 2):
                            ps_o0 = psO.tile([P, QT], F32, tag="psO", name="pso0")
                            ps_o1 = psO.tile([P, QT], F32, tag="psO", name="pso1")
                            for kt in range(KTC):
                                ps_s0 = psS.tile([P, QT], F32, tag="psS", name="pss0")
                                ps_s1 = psS.tile([P, QT], F32, tag="psS", name="pss1")
                                nc.tensor.matmul(
                                    ps_s0[:],
                                    lhsT=qk_t[0:64, 2 + pp, kt * P:(kt + 1) * P],
                                    rhs=qk_t[0:64, pp, q0:q0 + QT],
                                    start=True, stop=True)
                                nc.tensor.matmul(
                                    ps_s1[:],
                                    lhsT=qk_t[64:128, 2 + pp, kt * P:(kt + 1) * P],
                                    rhs=qk_t[64:128, pp, q0:q0 + QT],
                                    start=True, stop=True, tile_position=(64, 0))
                                e0 = expp.tile([P, QT], BF16, tag="exp", name="e0")
                                e1 = expp.tile([P, QT], BF16, tag="exp", name="e1")
                                nc.scalar.activation(e0[:], ps_s0[:], EXP)
                                nc.scalar.activation(e1[:], ps_s1[:], EXP)
                                # interleaved work fills the exp-wait window
                                slot += 1
                                for fg, f_from, f_every in feeders:
                                    if slot > f_from and \
                                            (slot - f_from) % f_every == 0:
                                        next(fg, None)
                                if slot % 2 == 0:
                                    next(pgen, None)
                                h0 = 2 * pp
                                h1 = 2 * pp + 1
                                nc.tensor.matmul(
                                    ps_o0[0:hd + 1, :],
                                    lhsT=v_t[:, kt, h0 * (hd + 1):(h0 + 1) * (hd + 1)],
                                    rhs=e0[:],
                                    start=(kt == 0), stop=(kt == KTC - 1))
                                nc.tensor.matmul(
                                    ps_o1[0:hd + 1, :],
                                    lhsT=v_t[:, kt, h1 * (hd + 1):(h1 + 1) * (hd + 1)],
                                    rhs=e1[:],
                                    start=(kt == 0), stop=(kt == KTC - 1))
                            # stash to SBUF, free psum
                            ao = aop.tile([hd + 1, 2, QT], F32, tag="ao", name="ao")
                            nc.vector.tensor_copy(ao[:, 0, :], ps_o0[0:hd + 1, :])
                            nc.vector.tensor_copy(ao[:, 1, :], ps_o1[0:hd + 1, :])
                            # softmax denominators -> 1/den, broadcast via DRAM
                            dg = dgp.tile([2, QT], F32, tag="dg", name="dg")
                            nc.sync.dma_start(dg[:], ao[hd:hd + 1, :, :])
                            rcp = dgp.tile([2, QT], F32, tag="rcp", name="rcp")
                            nc.vector.reciprocal_approx_fast(rcp[:], dg[:])
                            rd = drowp.tile([2, QT], F32, tag="drow", name="rd")
                            rep = repp.tile([hd, 2, QT], F32, tag="rep", name="rep")
                            # rd + broadcasts all on sync: same-FIFO keeps
                            # rd-write -> rep-read ordered (cross-queue DRAM
                            # deps raced); scalar stays clear for the exp
                            # cadence, which paces SDPA
                            nc.sync.dma_start(rd[:], rcp[:])
                            nc.sync.dma_start(rep[:, 0, :],
                                              rd[0:1, :].to_broadcast((hd, QT)))
                            nc.sync.dma_start(rep[:, 1, :],
                                              rd[1:2, :].to_broadcast((hd, QT)))
                            aon = aonp.tile([hd, 2, QT], BF16, tag="aon", name="aon")
                            nc.vector.tensor_tensor(aon[:], ao[0:hd, :, :], rep[:], MULT)
                            # scatter (one DMA per head)
                            if bi < b - 1:
                                j0 = qt * (QT // RT)
                                for h in range(2):
                                    nc.sync.dma_start(
                                        ccin_v[bi][pp, h][:, j0:j0 + QT // RT, :],
                                        aon[:, h, :].rearrange(
                                            "v (j t) -> v j t", t=RT))
                            else:
                                for h in range(2):
                                    nc.sync.dma_start(
                                        cc3in_v[qt][pp, h],
                                        aon[:, h, :].rearrange(
                                            "v (j t) -> v j t", t=RT // 2))
                        if bi == b - 1:  # fire this qt-half's exchange now
                            nc.gpsimd.collective_compute(
                                "AllToAll", mybir.AluOpType.bypass,
                                replica_groups=[list(range(n_cores))],
                                ins=[cc3_in[qt].opt()], outs=[cc3_out[qt].opt()])
                            # preload this half's o_proj input right behind
                            # the trigger: the NEXT half's trigger would
                            # otherwise block it on the gpsimd FIFO
                            nc.gpsimd.dma_start(asl3_t[qt][:], cc3out_v[qt])
                    if bi < b - 1:
                        a2a(bi)

                def empty_gen():
                    return iter(())

                hid_store[0] = hid_next
                asl3_t = [aslp.tile([P, ECH, RT // 2], BF16, tag="asl3",
                                    name=f"asl3_{q}") for q in range(NQT)]
                qk_t = qkp.tile([P, NQK, s], BF16, tag="qk", name="qk")
                v_t = vp.tile([P, KTC, EVA], BF16, tag="v", name="v")
                gen0 = proj_gen(0, qk_t, v_t)
                next(gen0, None)  # emit v ones-init on scalar queue first
                # w_o chunks trickle between proj steps so they don't
                # contend with hid/qk loads for HBM at startup (w_o is
                # first needed by o_proj r0, ~150us in)
                def wo_gen(dc0):
                    """Emit one w_o chunk per step; first needed ~255us in,
                    so most of it trickles through sdpa(b0)'s slots instead
                    of competing with the critical startup loads."""
                    for dc in range(dc0, 16):
                        nc.scalar.dma_start(
                            wo_sb[:, :, dc * 128:(dc + 1) * 128],
                            wo_v[:, :, dc * 128:(dc + 1) * 128])
                        yield

                dc = 0
                for _ in gen0:  # batch 0: run fully
                    if dc < 2:
                        nc.scalar.dma_start(
                            wo_sb[:, :, dc * 128:(dc + 1) * 128],
                            wo_v[:, :, dc * 128:(dc + 1) * 128])
                        dc += 1
                wog = wo_gen(2)

                # schedule: r0 o_proj fills sdpa(b1); r1 is deliberately
                # HELD OUT of sdpa(b2) (proj(b3) is enough fill there) and
                # interleaved into sdpa(b3) together with r2, because b3
                # has no projection left and would otherwise run at
                # exp-activation latency (~2.2us/iter vs 0.8us PE work).
                gens = {}
                for bi in range(b):
                    if bi + 1 < b:
                        qk_n = qkp.tile([P, NQK, s], BF16, tag="qk", name="qk")
                        v_n = vp.tile([P, KTC, EVA], BF16, tag="v", name="v")
                        pgen = proj_gen(bi + 1, qk_n, v_n)
                    else:
                        pgen = empty_gen()
                    if bi == 1:
                        sdpa(bi, qk_t, v_t, [(gens[0], 16, 2)], pgen)
                        for _ in gens[0]:  # finish r0
                            pass
                    elif bi == 3:
                        # r1 data has long landed; r2's A2A lands ~25us in,
                        # so gate it to slot >= 20. every-2 pacing on both
                        # keeps b3's sdpa near its exp-cadence floor and
                        # reserves ~20us of matmuls to cover the cc3 A2A
                        # latency windows after the loop
                        sdpa(bi, qk_t, v_t, [(gens[1], 0, 2), (gens[2], 20, 2)],
                             pgen)
                        for _ in gens[1]:
                            pass
                        for _ in gens[2]:  # drains into the cc3-q0 A2A wait
                            pass
                    else:
                        if bi == 0:
                            # remaining w_o chunks trickle through b0's
                            # slots (PE-bound here, scalar exp has slack)
                            sdpa(bi, qk_t, v_t, [(wog, 0, 2)], pgen)
                            for _ in wog:
                                pass
                        else:
                            sdpa(bi, qk_t, v_t, [], pgen)
                    for _ in pgen:  # drain leftover projection steps
                        pass
                    if bi < b - 1:
                        gens[bi] = oproj_gen(bi)
                        next(gens[bi], None)  # emit asl DMA (gpsimd) eagerly
                        qk_t, v_t = qk_n, v_n
                # last batch o_proj: per qt-half, first overlaps second's A2A
                def oproj3(q):
                    """Half-round o_proj with M=64: col-tile the two d-out
                    chunks onto partitions [0:64] and [64:128] so the pair
                    streams concurrently and the full PE array is used."""
                    asl3 = asl3_t[q]
                    r0 = (b - 1) * RT + q * (RT // 2)
                    for dcg in range(NDC // 2):
                        pj = psJ.tile([P, ODC], F32, tag="psJ", name="pj")
                        d0 = (2 * dcg) * ODC
                        d1 = (2 * dcg + 1) * ODC
                        for e in range(ECH):
                            nc.tensor.matmul(pj[0:RT // 2, :],
                                             lhsT=asl3[:, e, :],
                                             rhs=wo_sb[:, e, d0:d0 + ODC],
                                             start=(e == 0), stop=(e == ECH - 1))
                            nc.tensor.matmul(pj[64:64 + RT // 2, :],
                                             lhsT=asl3[:, e, :],
                                             rhs=wo_sb[:, e, d1:d1 + ODC],
                                             start=(e == 0), stop=(e == ECH - 1))
                        ob = obp.tile([P, ODC], F32, tag="ob3", name="ob3",
                                      bufs=2)
                        nc.vector.tensor_copy(ob[0:RT // 2, :], pj[0:RT // 2, :])
                        nc.vector.tensor_copy(ob[64:64 + RT // 2, :],
                                              pj[64:64 + RT // 2, :])
                        nc.scalar.dma_start(
                            out_sl[r0:r0 + RT // 2, d0:d0 + ODC],
                            ob[0:RT // 2, :])
                        nc.scalar.dma_start(
                            out_sl[r0:r0 + RT // 2, d1:d1 + ODC],
                            ob[64:64 + RT // 2, :])

                for q in range(NQT):
                    oproj3(q)
    nc.finalize()
    return nc


def prep_inputs(cos, sin, hidden_states, w_qkv, w_o,
                b=B, s=S, d=D, h_loc=H_LOC, hd=HD, n_cores=N_CORES):
    """Host-side sharding/layout: returns per-core input maps."""
    BF = ml_dtypes.bfloat16
    cos = np.asarray(cos, dtype=np.float32)
    sin = np.asarray(sin, dtype=np.float32)
    hidden_states = np.asarray(hidden_states, dtype=np.float32)
    w_qkv = np.asarray(w_qkv, dtype=np.float32)
    w_o = np.asarray(w_o, dtype=np.float32)

    T = b * s
    QBLK = h_loc * hd
    HHD = n_cores * QBLK  # total H*HD

    hidden_t = np.ascontiguousarray(hidden_states.reshape(T, d).T).astype(BF)
    w_o_t = np.ascontiguousarray(w_o.T).astype(BF)

    ident2_np = np.ascontiguousarray(
        np.vstack([np.eye(hd), np.eye(hd)]).astype(np.float32))
    cos_t = cos.T  # [hd, s]
    sin_t = sin.T
    cos2 = np.ascontiguousarray(np.tile(cos_t, (128 // hd, 1))).astype(BF)
    srt = sin_t.copy()
    srt[0:hd // 2] = -sin_t[0:hd // 2]
    sinrot2 = np.ascontiguousarray(np.tile(srt, (128 // hd, 1))).astype(BF)

    maps = []
    for c in range(n_cores):
        wq = w_qkv[c * QBLK:(c + 1) * QBLK] * 0.125
        wk = w_qkv[HHD + c * QBLK:HHD + (c + 1) * QBLK]
        wv = w_qkv[2 * HHD + c * QBLK:2 * HHD + (c + 1) * QBLK]
        w_qk_t = np.ascontiguousarray(np.concatenate([wq, wk], axis=0).T).astype(BF)
        w_v_t = np.ascontiguousarray(wv.T).astype(BF)
        maps.append({
            "hidden_t": hidden_t,
            "w_qk_t": w_qk_t,
            "w_v_t": w_v_t,
            "w_o_t": w_o_t,
            "cos2": cos2,
            "sinrot2": sinrot2,
        })
    return maps


_NC_CACHE = {}


def run(inputs, trace=False, dims=None):
    """Run the distributed kernel. Returns (full_output, BassKernelResults)."""
    dims = dims or dict(b=B, s=S, d=D, h_loc=H_LOC, hd=HD, n_cores=N_CORES)
    key = tuple(sorted(dims.items()))
    if key not in _NC_CACHE:
        _NC_CACHE[key] = build_attention(**dims)
    nc = _NC_CACHE[key]
    maps = prep_inputs(inputs["cos"], inputs["sin"], inputs["hidden_states"],
                       inputs["w_qkv"], inputs["w_o"], **dims)
    res = run_bass_kernel_spmd(nc, maps, list(range(dims["n_cores"])), trace=trace)
    n_cores = dims["n_cores"]
    s = dims["s"]
    RT = s // n_cores  # 128
    T = dims["b"] * s
    out = np.empty((T, dims["d"]), dtype=np.float32)
    for c in range(n_cores):
        sl = res.results[c]["out_sl"]
        for r in range(dims["b"] - 1):
            out[r * s + c * RT: r * s + (c + 1) * RT] = sl[r * RT:(r + 1) * RT]
    # last batch was exchanged in qt halves of 64 tokens per core
    r = dims["b"] - 1
    for c in range(n_cores):
        sl = res.results[c]["out_sl"]
        for q in range(2):
            g0 = r * s + q * 512 + c * (RT // 2)
            o0 = r * RT + q * (RT // 2)
            out[g0:g0 + RT // 2] = sl[o0:o0 + RT // 2]
    out = out.reshape(dims["b"], s, dims["d"])
    return out, res


def kernel(**inputs) -> np.ndarray:
    out, _ = run(inputs)
    return out

